# revision 32
# baseline (speedup 1.0000x reference)
"""ChebyASPIRE spectral filter on 8 TRN2 NeuronCores.

Algorithm (Gram-matrix formulation):
  phase 1: Z = X^T X  (4096x4096), column-sharded: core i computes
           Z[:, i*512:(i+1)*512] from a full stream of X (fp16 operands,
           fp32 PSUM accumulate), kept in SBUF as fp16.
  phase 2: Chebyshev recurrence t_k = 2*Zs t_{k-1} - t_{k-2} with
           Zs = (Z - t_mid I)/t_half, applied to V = R^T (4096x256).
           Row-sharded: core i computes rows [i*512, (i+1)*512) of each
           t_k using lhsT = Z[:, ib] (== Z[ib, :]^T by symmetry), then
           AllGathers the new t shard so every core has the full t for
           the next step.  The batch (256 query users) is split into two
           halves that advance in lockstep but alternate on the PE, so
           each half's AllGather hides under the other half's matmuls.

           The gathered shard transits as int8 (exactly 64KB per core:
           the AllGather algorithm crosses over at 64KB input / 512KB
           output and jumps from ~7us to ~11us, so nothing beyond the
           payload may travel).  Quant scales are SELF-SYNCHRONIZED:
           every core derives the step-s scale deterministically from
           the step s-1 gathered payload (per-partition max of the
           dequantized values x GM margin), so sender and receivers
           agree bit-exactly without transmitting scales.  Only the
           s=1 bootstrap AllGather carries explicit per-partition fp32
           scales (66048B, one-time slow AG).  Receivers dequantize to
           fp16 on the vector engine; recurrence state and accumulator
           stay fp32.

Inputs come in full; sharding/layout prep happens on host.  Scalars
(t_mid, t_half, coeffs) are baked into the program as immediates; the
program is rebuilt (and NEFF-cached) per distinct scalar set.
"""
import sys

sys.path.insert(0, "/opt/trn_rl_repo")

import numpy as np

M, N, B = 8192, 4096, 256
NC = 8
CB = N // NC          # 512 columns/rows per core
DEG = 20              # Chebyshev degree (21 coeffs)
KT1 = M // 128        # 64 k-tiles in phase 1
MP1 = N // 128        # 32 m-passes in phase 1
KT2 = N // 128        # 32 k-tiles in phase 2
MS2 = CB // 128       # 4 m-subs in phase 2
NH = 2                # batch halves in phase 2
BH = B // NH          # 128 columns per half
QMAX = 126.5          # int8 quant headroom (safe under any rounding)
GM = 4.0              # scale growth margin: |t_s| <= 2|Zs t_{s-1}|+|t_{s-2}|
PAY = MS2 * BH        # 512 int8 payload bytes per partition line
LINE = PAY + 4        # + fp32 scale -> 516 bytes (s=1 bootstrap only)

_BUILD_CACHE = {}


def _build(scalars):
    """Build the SPMD Bass program for the given (t_mid, t_half, *coeffs)."""
    from concourse import bacc, tile, mybir

    tm, th = scalars[0], scalars[1]
    c = scalars[2:]
    f16 = mybir.dt.float16
    f32 = mybir.dt.float32
    i8 = mybir.dt.int8
    mult = mybir.AluOpType.mult
    add = mybir.AluOpType.add
    sub = mybir.AluOpType.subtract
    amax = mybir.AluOpType.max

    nc = bacc.Bacc("TRN2", target_bir_lowering=False, debug=False,
                   num_devices=NC)
    Xh = nc.dram_tensor("X16", [M, N], f16, kind="ExternalInput")
    Xb = nc.dram_tensor("Xblk16", [M, CB], f16, kind="ExternalInput")
    Vh = nc.dram_tensor("V16", [N, B], f16, kind="ExternalInput")
    Vb = nc.dram_tensor("Vblk32", [CB, B], f32, kind="ExternalInput")
    acc_out = nc.dram_tensor("acc_out", [CB, B], f32, kind="ExternalOutput")

    RCH = 4                      # k-tiles per rhs_res chunk (phase 1)
    LCH = 16                     # k-tiles per lh chunk (phase 1)
    with tile.TileContext(nc) as tc:
        with (
            tc.tile_pool(name="persist", bufs=1) as persist,
            tc.tile_pool(name="dram", bufs=1, space="DRAM") as dram,
        ):
            zk = [persist.tile([128, CB], f16, name=f"zk{i}")
                  for i in range(KT2)]

            # warmup collectives: pay first-call cost + warm both AG
            # sizes used in phase 2, while phase 1 computes (no one
            # consumes the results)
            w_outs = []
            for wi, wb in enumerate((64 * LINE, 64 * PAY)):
                w_in = dram.tile([wb], f16, name=f"w_in{wi}")
                w_out = dram.tile([wb * NC], f16, addr_space="Shared",
                                  name=f"w_out{wi}")
                nc.sync.dma_start(
                    w_in[:],
                    Vh[0:wb // 256, :].rearrange("a b -> (a b)"))
                nc.gpsimd.collective_compute(
                    "AllGather", mybir.AluOpType.bypass,
                    replica_groups=[list(range(NC))],
                    ins=[w_in[:]], outs=[w_out[:]])
                w_outs.append(w_out)

            # ---------------- phase 1: Z[:, ib] = X^T X[:, ib] -------------
            with (
                tc.tile_pool(name="rhsres", bufs=1) as rhsres,
                tc.tile_pool(name="lstream", bufs=3) as lstream,
                tc.tile_pool(name="ps1", bufs=2, space="PSUM") as ps1,
            ):
                # resident rhs X[:, ib], chunked so matmuls start early
                rhs_res = [rhsres.tile([128, RCH, CB], f16,
                                       name=f"rhs_res{cc}")
                           for cc in range(KT1 // RCH)]
                Xb3 = Xb[:, :].rearrange("(kk p) cb -> p kk cb", p=128)
                # only the first chunks before the first lhs stream; the
                # rest are issued after mp=0's lhs DMAs so the first
                # matmul isn't stuck behind 8MB of resident-rhs traffic.
                NPRE = 6
                for cc in range(NPRE):
                    nc.sync.dma_start(rhs_res[cc][:],
                                      Xb3[:, cc * RCH:(cc + 1) * RCH, :])

                for mp in range(MP1):
                    lhs = [lstream.tile([128, LCH, 128], f16, name=f"lh{h}")
                           for h in range(KT1 // LCH)]
                    Xm3 = (Xh[:, mp * 128:(mp + 1) * 128]
                           .rearrange("(kk p) mc -> p kk mc", p=128))
                    if mp == 5:
                        # staged alignment gate: by now warmup AG #0 has
                        # completed on every core; blocking mp=5's lhs
                        # load on it aligns the PE streams at ~zero cost
                        # so the bootstrap AllGather absorbs no skew.
                        gate = rhsres.tile([128, 1, 1], f16, name="gate")
                        nc.sync.dma_start(
                            gate[:],
                            w_outs[0][0:128].rearrange(
                                "(p a b) -> p a b", p=128, a=1))
                        nc.vector.tensor_copy(lhs[0][:, 0:1, 0:1],
                                              gate[:])
                    for h in range(KT1 // LCH):
                        nc.sync.dma_start(lhs[h][:],
                                          Xm3[:, h * LCH:(h + 1) * LCH, :])
                    if mp == 0:
                        for cc in range(NPRE, KT1 // RCH):
                            nc.sync.dma_start(
                                rhs_res[cc][:],
                                Xb3[:, cc * RCH:(cc + 1) * RCH, :])
                    zps = ps1.tile([128, CB], f32, name="zps")
                    for kk in range(KT1):
                        nc.tensor.matmul(
                            zps[:],
                            lhs[kk // LCH][:, kk % LCH, :],
                            rhs_res[kk // RCH][:, kk % RCH, :],
                            start=(kk == 0), stop=(kk == KT1 - 1))
                    nc.vector.tensor_copy(zk[mp][:], zps[:])

            # ---------------- phase 2: Chebyshev recurrence ----------------
            with (
                tc.tile_pool(name="rhsp", bufs=2) as rhsp,
                tc.tile_pool(name="qpool", bufs=2) as qpool,
                tc.tile_pool(name="dvep", bufs=4) as dvep,
                tc.tile_pool(name="stagep", bufs=2) as stagep,
                tc.tile_pool(name="ps2", bufs=2, space="PSUM") as ps2p,
            ):
                # per-half fp32 state shards (128 part x [4 m-subs x 128])
                tstate = [[persist.tile([128, MS2, BH], f32,
                                        name=f"tst{h}_{i}")
                           for i in range(3)] for h in range(NH)]
                acc = [persist.tile([128, MS2, BH], f32, name=f"acc{h}")
                       for h in range(NH)]
                # self-synchronized quant-scale state, per half:
                # rinvS = QMAX/(GM*M), scdeqS = GM*M/QMAX  with M the
                # per-partition max |dequantized t_{s-1}| every core
                # computes identically from the gathered payload.
                rinvS = [persist.tile([128, 1], f32, name=f"rinvS{h}")
                         for h in range(NH)]
                scdeqS = [persist.tile([128, 1], f32, name=f"scdeqS{h}")
                          for h in range(NH)]
                Vb3 = Vb[:, :].rearrange("(ms p) b -> p ms b", p=128)
                for h in range(NH):
                    nc.sync.dma_start(tstate[h][0][:],
                                      Vb3[:, :, h * BH:(h + 1) * BH])

                Vh3 = Vh[:, :].rearrange("(kk p) b -> p kk b", p=128)
                agout = [[None] * NH for _ in range(DEG)]

                for s in range(1, DEG + 1):
                    for h in range(NH):
                        # rhs: full t_{s-1} half (4096 x 128) fp16
                        rh = rhsp.tile([128, KT2, BH], f16, name=f"rh{h}")
                        if s == 1:
                            for j in range(NC):
                                nc.sync.dma_start(
                                    rh[:, j * RCH:(j + 1) * RCH, :],
                                    Vh3[:, j * RCH:(j + 1) * RCH,
                                        h * BH:(h + 1) * BH])
                        elif s == 2:
                            # bootstrap payload: per-j fp32 scales ride
                            # in the 516-byte lines
                            src = (agout[0][h][:]
                                   .rearrange("(j p l) -> p j l",
                                              j=NC, p=128))
                            comb = qpool.tile([128, NC, LINE], i8,
                                              name=f"comb5{h}")
                            nc.scalar.dma_start(comb[:, 0:1, :],
                                                src[:, 0:1, :])
                            nc.scalar.dma_start(comb[:, 1:4, :],
                                                src[:, 1:4, :])
                            nc.scalar.dma_start(comb[:, 4:8, :],
                                                src[:, 4:8, :])
                            for j in range(NC):
                                nc.scalar.activation(
                                    rh[:, j * RCH:(j + 1) * RCH, :],
                                    comb[:, j, 0:PAY].rearrange(
                                        "p (ms b) -> p ms b", ms=MS2),
                                    mybir.ActivationFunctionType.Copy,
                                    scale=comb[:, j, PAY:LINE]
                                    .bitcast(f32))
                        else:
                            src = (agout[s - 2][h][:]
                                   .rearrange("(j p l) -> p j l",
                                              j=NC, l=PAY))
                            comb = qpool.tile([128, NC, PAY], i8,
                                              name=f"comb{h}")
                            nc.scalar.dma_start(comb[:, 0:1, :],
                                                src[:, 0:1, :])
                            nc.scalar.dma_start(comb[:, 1:4, :],
                                                src[:, 1:4, :])
                            nc.scalar.dma_start(comb[:, 4:8, :],
                                                src[:, 4:8, :])
                            for j in range(NC):
                                nc.scalar.activation(
                                    rh[:, j * RCH:(j + 1) * RCH, :],
                                    comb[:, j, :].rearrange(
                                        "p (ms b) -> p ms b", ms=MS2),
                                    mybir.ActivationFunctionType.Copy,
                                    scale=scdeqS[h][:])

                        if 2 <= s < DEG:
                            # next-step scale, identical on every core:
                            # M = max|dequant| = max|q| * scdeq
                            mxt = dvep.tile([128, 1], f32, name=f"mx{h}")
                            if s == 2:
                                # one-time: per-j scales, reduce dequant
                                nc.vector.tensor_reduce(
                                    mxt[:], rh[:],
                                    axis=mybir.AxisListType.XY,
                                    op=amax, apply_absolute_value=True)
                            else:
                                # int8 payload reduce: starts right after
                                # the comb DMA, done before the u chain
                                nc.vector.tensor_reduce(
                                    mxt[:], comb[:],
                                    axis=mybir.AxisListType.XY,
                                    op=amax, apply_absolute_value=True)
                                nc.vector.tensor_scalar_mul(
                                    mxt[:], mxt[:], scdeqS[h][:])
                            nc.vector.tensor_scalar_max(mxt[:], mxt[:],
                                                        1e-30)
                            nc.vector.reciprocal(rinvS[h][:], mxt[:])
                            nc.vector.tensor_scalar_mul(
                                rinvS[h][:], rinvS[h][:], QMAX / GM)
                            nc.vector.tensor_scalar_mul(
                                scdeqS[h][:], mxt[:], GM / QMAX)

                        Tc = tstate[h][(s - 1) % 3]
                        Tp = tstate[h][(s - 2) % 3] if s >= 2 else None
                        Tn = tstate[h][s % 3]
                        ach = acc[h]

                        u = dvep.tile([128, MS2, BH], f32, name=f"u{h}")
                        # one PSUM tile spanning 4 banks: quarter ms
                        # lives at bank offset ms*512 f32, so the four
                        # accumulation groups are in distinct banks and
                        # the epilogue reads all of W in single wide ops
                        wps = ps2p.tile([128, MS2, 512], f32, name="wps")
                        wv = wps[:, :, 0:BH]
                        # kk-outer so early k-tiles (first arrived rh
                        # slabs) start matmuls before later ones land
                        for kk in range(KT2):
                            for ms in range(MS2):
                                nc.tensor.matmul(
                                    wps[:, ms, 0:BH],
                                    zk[kk][:, ms * 128:(ms + 1) * 128],
                                    rh[:, kk, :],
                                    start=(kk == 0), stop=(kk == KT2 - 1))
                        # u = W - tm * Tc  (single op across banks)
                        nc.vector.scalar_tensor_tensor(
                            u[:], Tc[:], -tm, wv, op0=mult, op1=add)
                        if s == 1:
                            # T1 = u / th ;  acc = c0*T0 + c1*T1
                            nc.vector.tensor_scalar_mul(
                                Tn[:], u[:], 1.0 / th)
                            nc.vector.tensor_scalar_mul(
                                ach[:], Tn[:], c[1])
                            nc.vector.scalar_tensor_tensor(
                                ach[:], Tc[:], c[0], ach[:],
                                op0=mult, op1=add)
                        else:
                            # Tn -> quant -> (deferred) acc
                            nc.vector.scalar_tensor_tensor(
                                Tn[:], u[:], 2.0 / th, Tp[:],
                                op0=mult, op1=sub)
                            if s < DEG:
                                qs = stagep.tile([128, PAY], i8,
                                                 name=f"qs{h}")
                                nc.vector.tensor_scalar_mul(
                                    qs[:, :],
                                    Tn[:].rearrange("p ms b -> p (ms b)"),
                                    rinvS[h][:])
                            nc.vector.scalar_tensor_tensor(
                                ach[:], Tn[:], c[s], ach[:],
                                op0=mult, op1=add)

                        if s < DEG:
                            if s == 1:
                                # bootstrap: exact per-partition scale,
                                # transmitted in-line (one-time 66KB AG)
                                mx = dvep.tile([128, 1], f32,
                                               name=f"mxb{h}")
                                ri = dvep.tile([128, 1], f32,
                                               name=f"rib{h}")
                                qs = stagep.tile([128, LINE], i8,
                                                 name=f"qsb{h}")
                                nc.vector.tensor_reduce(
                                    mx[:], Tn[:],
                                    axis=mybir.AxisListType.XY,
                                    op=amax, apply_absolute_value=True)
                                nc.vector.tensor_scalar_max(
                                    mx[:], mx[:], 1e-30)
                                nc.vector.reciprocal(ri[:], mx[:])
                                nc.vector.tensor_scalar_mul(
                                    ri[:], ri[:], QMAX)
                                nc.vector.tensor_scalar_mul(
                                    qs[:, 0:PAY],
                                    Tn[:].rearrange("p ms b -> p (ms b)"),
                                    ri[:])
                                nc.vector.tensor_scalar_mul(
                                    qs[:, PAY:LINE].bitcast(f32), mx[:],
                                    1.0 / QMAX)
                                nbytes = LINE
                            else:
                                nbytes = PAY

                            agin = dram.tile([128 * nbytes], i8,
                                             name=f"agin{s}_{h}")
                            nc.gpsimd.dma_start(
                                agin[:].rearrange("(p l) -> p l", p=128),
                                qs[:])
                            agout[s - 1][h] = dram.tile(
                                [NC * 128 * nbytes], i8,
                                addr_space="Shared",
                                name=f"agout{s}_{h}")
                            nc.gpsimd.collective_compute(
                                "AllGather",
                                mybir.AluOpType.bypass,
                                replica_groups=[list(range(NC))],
                                ins=[agin[:]],
                                outs=[agout[s - 1][h][:]],
                            )

                out3 = acc_out[:, :].rearrange("(ms p) b -> p ms b", p=128)
                for h in range(NH):
                    nc.sync.dma_start(out3[:, :, h * BH:(h + 1) * BH],
                                      acc[h][:])

    nc.finalize()
    return nc


def _get_program(scalars):
    key = tuple(np.asarray(scalars, np.float64).tolist())
    if key not in _BUILD_CACHE:
        _BUILD_CACHE[key] = _build(key)
    return _BUILD_CACHE[key]


def _run(X, R, coeffs, t_mid, t_half, trace=False):
    from concourse.bass_utils import run_bass_kernel_spmd

    X = np.ascontiguousarray(np.asarray(X, np.float32))
    R = np.ascontiguousarray(np.asarray(R, np.float32))
    coeffs = np.asarray(coeffs, np.float32)
    tm = float(np.asarray(t_mid).reshape(-1)[0])
    th = float(np.asarray(t_half).reshape(-1)[0])

    nc = _get_program((tm, th, *[float(v) for v in coeffs]))

    X16 = X.astype(np.float16)
    V32 = np.ascontiguousarray(R.T.astype(np.float32))   # (N, B)
    V16 = V32.astype(np.float16)

    in_maps = []
    for i in range(NC):
        ib = slice(i * CB, (i + 1) * CB)
        in_maps.append({
            "X16": X16,
            "Xblk16": np.ascontiguousarray(X16[:, ib]),
            "V16": V16,
            "Vblk32": np.ascontiguousarray(V32[ib, :]),
        })

    res = run_bass_kernel_spmd(nc, in_maps, core_ids=list(range(NC)),
                               trace=trace)

    out = np.empty((B, N), np.float32)
    for i in range(NC):
        out[:, i * CB:(i + 1) * CB] = res.results[i]["acc_out"].T
    return out, res


def kernel(X, R, coeffs, t_mid, t_half):
    out, _ = _run(X, R, coeffs, t_mid, t_half, trace=False)
    return out


# revision 33
# speedup vs baseline: 1.0913x; 1.0913x over previous
"""ChebyASPIRE spectral filter on 8 TRN2 NeuronCores.

Algorithm (Gram-matrix formulation):
  phase 1: Z = X^T X  (4096x4096), column-sharded: core i computes
           Z[:, i*512:(i+1)*512] from a full stream of X (fp16 operands,
           fp32 PSUM accumulate), kept in SBUF as fp16.
  phase 2: Chebyshev recurrence t_k = 2*Zs t_{k-1} - t_{k-2} with
           Zs = (Z - t_mid I)/t_half, applied to V = R^T (4096x256).
           Row-sharded: core i computes rows [i*512, (i+1)*512) of each
           t_k using lhsT = Z[:, ib] (== Z[ib, :]^T by symmetry), then
           AllGathers the new t shard so every core has the full t for
           the next step.  The batch (256 query users) is split into two
           halves that advance in lockstep but alternate on the PE, so
           each half's AllGather hides under the other half's matmuls.

           The gathered shard transits as int8 (exactly 64KB per core:
           the AllGather algorithm crosses over at 64KB input / 512KB
           output and jumps from ~7us to ~11us, so nothing beyond the
           payload may travel).  Quant scales are SELF-SYNCHRONIZED:
           every core derives the step-s scale deterministically from
           the step s-1 gathered payload (per-partition max of the
           dequantized values x GM margin), so sender and receivers
           agree bit-exactly without transmitting scales.  Only the
           s=1 bootstrap AllGather carries explicit per-partition fp32
           scales (66048B, one-time slow AG).  Receivers dequantize to
           fp16 on the vector engine; recurrence state and accumulator
           stay fp32.

Inputs come in full; sharding/layout prep happens on host.  Scalars
(t_mid, t_half, coeffs) are baked into the program as immediates; the
program is rebuilt (and NEFF-cached) per distinct scalar set.
"""
import sys

sys.path.insert(0, "/opt/trn_rl_repo")

import numpy as np

M, N, B = 8192, 4096, 256
NC = 8
CB = N // NC          # 512 columns/rows per core
DEG = 20              # Chebyshev degree (21 coeffs)
KT1 = M // 128        # 64 k-tiles in phase 1
MP1 = N // 128        # 32 m-passes in phase 1
KT2 = N // 128        # 32 k-tiles in phase 2
MS2 = CB // 128       # 4 m-subs in phase 2
NH = 2                # batch halves in phase 2
BH = B // NH          # 128 columns per half
QMAX = 126.5          # int8 quant headroom (safe under any rounding)
GM = 4.0              # scale growth margin: |t_s| <= 2|Zs t_{s-1}|+|t_{s-2}|
PAY = MS2 * BH        # 512 int8 payload bytes per partition line
LINE = PAY + 4        # + fp32 scale -> 516 bytes (s=1 bootstrap only)

_BUILD_CACHE = {}


def _build(scalars):
    """Build the SPMD Bass program for the given (t_mid, t_half, *coeffs)."""
    from concourse import bacc, tile, mybir

    tm, th = scalars[0], scalars[1]
    c = scalars[2:]
    f16 = mybir.dt.float16
    f32 = mybir.dt.float32
    i8 = mybir.dt.int8
    mult = mybir.AluOpType.mult
    add = mybir.AluOpType.add
    sub = mybir.AluOpType.subtract
    amax = mybir.AluOpType.max

    nc = bacc.Bacc("TRN2", target_bir_lowering=False, debug=False,
                   num_devices=NC)
    Xh = nc.dram_tensor("X16", [M, N], f16, kind="ExternalInput")
    Xb = nc.dram_tensor("Xblk16", [M, CB], f16, kind="ExternalInput")
    Vh = nc.dram_tensor("V16", [N, B], f16, kind="ExternalInput")
    Vb = nc.dram_tensor("Vblk32", [CB, B], f32, kind="ExternalInput")
    acc_out = nc.dram_tensor("acc_out", [CB, B], f32, kind="ExternalOutput")

    RCH = 4                      # k-tiles per rhs_res chunk (phase 1)
    LCH = 16                     # k-tiles per lh chunk (phase 1)
    with tile.TileContext(nc) as tc:
        with (
            tc.tile_pool(name="persist", bufs=1) as persist,
            tc.tile_pool(name="dram", bufs=1, space="DRAM") as dram,
        ):
            zk = [persist.tile([128, CB], f16, name=f"zk{i}")
                  for i in range(KT2)]

            # warmup collectives: pay first-call cost + warm both AG
            # sizes used in phase 2, while phase 1 computes (no one
            # consumes the results)
            w_outs = []
            for wi, wb in enumerate((64 * LINE, 64 * PAY)):
                w_in = dram.tile([wb], f16, name=f"w_in{wi}")
                w_out = dram.tile([wb * NC], f16, addr_space="Shared",
                                  name=f"w_out{wi}")
                nc.sync.dma_start(
                    w_in[:],
                    Vh[0:wb // 256, :].rearrange("a b -> (a b)"))
                nc.gpsimd.collective_compute(
                    "AllGather", mybir.AluOpType.bypass,
                    replica_groups=[list(range(NC))],
                    ins=[w_in[:]], outs=[w_out[:]])
                w_outs.append(w_out)

            # ---------------- phase 1: Z[:, ib] = X^T X[:, ib] -------------
            with (
                tc.tile_pool(name="rhsres", bufs=1) as rhsres,
                tc.tile_pool(name="lstream", bufs=3) as lstream,
                tc.tile_pool(name="ps1", bufs=2, space="PSUM") as ps1,
            ):
                # resident rhs X[:, ib], chunked so matmuls start early
                rhs_res = [rhsres.tile([128, RCH, CB], f16,
                                       name=f"rhs_res{cc}")
                           for cc in range(KT1 // RCH)]
                Xb3 = Xb[:, :].rearrange("(kk p) cb -> p kk cb", p=128)
                # only the first chunks before the first lhs stream; the
                # rest are issued after mp=0's lhs DMAs so the first
                # matmul isn't stuck behind 8MB of resident-rhs traffic.
                NPRE = 6
                for cc in range(NPRE):
                    nc.sync.dma_start(rhs_res[cc][:],
                                      Xb3[:, cc * RCH:(cc + 1) * RCH, :])

                for mp in range(MP1):
                    lhs = [lstream.tile([128, LCH, 128], f16, name=f"lh{h}")
                           for h in range(KT1 // LCH)]
                    Xm3 = (Xh[:, mp * 128:(mp + 1) * 128]
                           .rearrange("(kk p) mc -> p kk mc", p=128))
                    for h in range(KT1 // LCH):
                        nc.sync.dma_start(lhs[h][:],
                                          Xm3[:, h * LCH:(h + 1) * LCH, :])
                    if mp == 0:
                        for cc in range(NPRE, KT1 // RCH):
                            nc.sync.dma_start(
                                rhs_res[cc][:],
                                Xb3[:, cc * RCH:(cc + 1) * RCH, :])
                    zps = ps1.tile([128, CB], f32, name="zps")
                    for kk in range(KT1):
                        nc.tensor.matmul(
                            zps[:],
                            lhs[kk // LCH][:, kk % LCH, :],
                            rhs_res[kk // RCH][:, kk % RCH, :],
                            start=(kk == 0), stop=(kk == KT1 - 1))
                    nc.vector.tensor_copy(zk[mp][:], zps[:])

            # ---------------- phase 2: Chebyshev recurrence ----------------
            with (
                tc.tile_pool(name="rhsp", bufs=2) as rhsp,
                tc.tile_pool(name="qpool", bufs=2) as qpool,
                tc.tile_pool(name="dvep", bufs=4) as dvep,
                tc.tile_pool(name="stagep", bufs=2) as stagep,
                tc.tile_pool(name="ps2", bufs=2, space="PSUM") as ps2p,
            ):
                # per-half fp32 state shards (128 part x [4 m-subs x 128])
                tstate = [[persist.tile([128, MS2, BH], f32,
                                        name=f"tst{h}_{i}")
                           for i in range(3)] for h in range(NH)]
                acc = [persist.tile([128, MS2, BH], f32, name=f"acc{h}")
                       for h in range(NH)]
                # self-synchronized quant-scale state, per half:
                # rinvS = QMAX/(GM*M), scdeqS = GM*M/QMAX  with M the
                # per-partition max |dequantized t_{s-1}| every core
                # computes identically from the gathered payload.
                rinvS = [persist.tile([128, 1], f32, name=f"rinvS{h}")
                         for h in range(NH)]
                scdeqS = [persist.tile([128, 1], f32, name=f"scdeqS{h}")
                          for h in range(NH)]
                Vb3 = Vb[:, :].rearrange("(ms p) b -> p ms b", p=128)
                for h in range(NH):
                    nc.sync.dma_start(tstate[h][0][:],
                                      Vb3[:, :, h * BH:(h + 1) * BH])

                Vh3 = Vh[:, :].rearrange("(kk p) b -> p kk b", p=128)
                agout = [[None] * NH for _ in range(DEG)]

                for s in range(1, DEG + 1):
                    for h in range(NH):
                        # rhs: full t_{s-1} half (4096 x 128) fp16
                        rh = rhsp.tile([128, KT2, BH], f16, name=f"rh{h}")
                        if s == 1:
                            for j in range(NC):
                                nc.sync.dma_start(
                                    rh[:, j * RCH:(j + 1) * RCH, :],
                                    Vh3[:, j * RCH:(j + 1) * RCH,
                                        h * BH:(h + 1) * BH])
                        elif s == 2:
                            # bootstrap payload: per-j fp32 scales ride
                            # in the 516-byte lines
                            src = (agout[0][h][:]
                                   .rearrange("(j p l) -> p j l",
                                              j=NC, p=128))
                            comb = qpool.tile([128, NC, LINE], i8,
                                              name=f"comb5{h}")
                            nc.scalar.dma_start(comb[:, 0:1, :],
                                                src[:, 0:1, :])
                            nc.scalar.dma_start(comb[:, 1:4, :],
                                                src[:, 1:4, :])
                            nc.scalar.dma_start(comb[:, 4:8, :],
                                                src[:, 4:8, :])
                            for j in range(NC):
                                nc.scalar.activation(
                                    rh[:, j * RCH:(j + 1) * RCH, :],
                                    comb[:, j, 0:PAY].rearrange(
                                        "p (ms b) -> p ms b", ms=MS2),
                                    mybir.ActivationFunctionType.Copy,
                                    scale=comb[:, j, PAY:LINE]
                                    .bitcast(f32))
                        else:
                            src = (agout[s - 2][h][:]
                                   .rearrange("(j p l) -> p j l",
                                              j=NC, l=PAY))
                            comb = qpool.tile([128, NC, PAY], i8,
                                              name=f"comb{h}")
                            nc.scalar.dma_start(comb[:, 0:1, :],
                                                src[:, 0:1, :])
                            nc.scalar.dma_start(comb[:, 1:4, :],
                                                src[:, 1:4, :])
                            nc.scalar.dma_start(comb[:, 4:8, :],
                                                src[:, 4:8, :])
                            for j in range(NC):
                                nc.scalar.activation(
                                    rh[:, j * RCH:(j + 1) * RCH, :],
                                    comb[:, j, :].rearrange(
                                        "p (ms b) -> p ms b", ms=MS2),
                                    mybir.ActivationFunctionType.Copy,
                                    scale=scdeqS[h][:])

                        if 2 <= s < DEG:
                            # next-step scale, identical on every core:
                            # M = max|dequant| = max|q| * scdeq
                            mxt = dvep.tile([128, 1], f32, name=f"mx{h}")
                            if s == 2:
                                # one-time: per-j scales, reduce dequant
                                nc.vector.tensor_reduce(
                                    mxt[:], rh[:],
                                    axis=mybir.AxisListType.XY,
                                    op=amax, apply_absolute_value=True)
                            else:
                                # int8 payload reduce: starts right after
                                # the comb DMA, done before the u chain
                                nc.vector.tensor_reduce(
                                    mxt[:], comb[:],
                                    axis=mybir.AxisListType.XY,
                                    op=amax, apply_absolute_value=True)
                                nc.vector.tensor_scalar_mul(
                                    mxt[:], mxt[:], scdeqS[h][:])
                            nc.vector.tensor_scalar_max(mxt[:], mxt[:],
                                                        1e-30)
                            nc.vector.reciprocal(rinvS[h][:], mxt[:])
                            nc.vector.tensor_scalar_mul(
                                rinvS[h][:], rinvS[h][:], QMAX / GM)
                            nc.vector.tensor_scalar_mul(
                                scdeqS[h][:], mxt[:], GM / QMAX)

                        Tc = tstate[h][(s - 1) % 3]
                        Tp = tstate[h][(s - 2) % 3] if s >= 2 else None
                        Tn = tstate[h][s % 3]
                        ach = acc[h]

                        u = dvep.tile([128, MS2, BH], f32, name=f"u{h}")
                        # one PSUM tile spanning 4 banks: quarter ms
                        # lives at bank offset ms*512 f32, so the four
                        # accumulation groups are in distinct banks and
                        # the epilogue reads all of W in single wide ops
                        wps = ps2p.tile([128, MS2, 512], f32, name="wps")
                        wv = wps[:, :, 0:BH]
                        # kk-outer so early k-tiles (first arrived rh
                        # slabs) start matmuls before later ones land
                        for kk in range(KT2):
                            for ms in range(MS2):
                                nc.tensor.matmul(
                                    wps[:, ms, 0:BH],
                                    zk[kk][:, ms * 128:(ms + 1) * 128],
                                    rh[:, kk, :],
                                    start=(kk == 0), stop=(kk == KT2 - 1))
                        # u = W - tm * Tc  (single op across banks)
                        nc.vector.scalar_tensor_tensor(
                            u[:], Tc[:], -tm, wv, op0=mult, op1=add)
                        if s == 1:
                            # T1 = u / th ;  acc = c0*T0 + c1*T1
                            nc.vector.tensor_scalar_mul(
                                Tn[:], u[:], 1.0 / th)
                            nc.vector.tensor_scalar_mul(
                                ach[:], Tn[:], c[1])
                            nc.vector.scalar_tensor_tensor(
                                ach[:], Tc[:], c[0], ach[:],
                                op0=mult, op1=add)
                        else:
                            # Tn -> quant -> (deferred) acc
                            nc.vector.scalar_tensor_tensor(
                                Tn[:], u[:], 2.0 / th, Tp[:],
                                op0=mult, op1=sub)
                            if s < DEG:
                                qs = stagep.tile([128, PAY], i8,
                                                 name=f"qs{h}")
                                nc.vector.tensor_scalar_mul(
                                    qs[:, :],
                                    Tn[:].rearrange("p ms b -> p (ms b)"),
                                    rinvS[h][:])
                            nc.vector.scalar_tensor_tensor(
                                ach[:], Tn[:], c[s], ach[:],
                                op0=mult, op1=add)

                        if s < DEG:
                            if s == 1:
                                # bootstrap: exact per-partition scale,
                                # transmitted in-line (one-time 66KB AG)
                                mx = dvep.tile([128, 1], f32,
                                               name=f"mxb{h}")
                                ri = dvep.tile([128, 1], f32,
                                               name=f"rib{h}")
                                qs = stagep.tile([128, LINE], i8,
                                                 name=f"qsb{h}")
                                nc.vector.tensor_reduce(
                                    mx[:], Tn[:],
                                    axis=mybir.AxisListType.XY,
                                    op=amax, apply_absolute_value=True)
                                nc.vector.tensor_scalar_max(
                                    mx[:], mx[:], 1e-30)
                                nc.vector.reciprocal(ri[:], mx[:])
                                nc.vector.tensor_scalar_mul(
                                    ri[:], ri[:], QMAX)
                                nc.vector.tensor_scalar_mul(
                                    qs[:, 0:PAY],
                                    Tn[:].rearrange("p ms b -> p (ms b)"),
                                    ri[:])
                                nc.vector.tensor_scalar_mul(
                                    qs[:, PAY:LINE].bitcast(f32), mx[:],
                                    1.0 / QMAX)
                                nbytes = LINE
                            else:
                                nbytes = PAY

                            agin = dram.tile([128 * nbytes], i8,
                                             name=f"agin{s}_{h}")
                            nc.gpsimd.dma_start(
                                agin[:].rearrange("(p l) -> p l", p=128),
                                qs[:])
                            agout[s - 1][h] = dram.tile(
                                [NC * 128 * nbytes], i8,
                                addr_space="Shared",
                                name=f"agout{s}_{h}")
                            nc.gpsimd.collective_compute(
                                "AllGather",
                                mybir.AluOpType.bypass,
                                replica_groups=[list(range(NC))],
                                ins=[agin[:]],
                                outs=[agout[s - 1][h][:]],
                            )

                out3 = acc_out[:, :].rearrange("(ms p) b -> p ms b", p=128)
                for h in range(NH):
                    nc.sync.dma_start(out3[:, :, h * BH:(h + 1) * BH],
                                      acc[h][:])

    nc.finalize()
    return nc


def _get_program(scalars):
    key = tuple(np.asarray(scalars, np.float64).tolist())
    if key not in _BUILD_CACHE:
        _BUILD_CACHE[key] = _build(key)
    return _BUILD_CACHE[key]


def _run(X, R, coeffs, t_mid, t_half, trace=False):
    from concourse.bass_utils import run_bass_kernel_spmd

    X = np.ascontiguousarray(np.asarray(X, np.float32))
    R = np.ascontiguousarray(np.asarray(R, np.float32))
    coeffs = np.asarray(coeffs, np.float32)
    tm = float(np.asarray(t_mid).reshape(-1)[0])
    th = float(np.asarray(t_half).reshape(-1)[0])

    nc = _get_program((tm, th, *[float(v) for v in coeffs]))

    X16 = X.astype(np.float16)
    V32 = np.ascontiguousarray(R.T.astype(np.float32))   # (N, B)
    V16 = V32.astype(np.float16)

    in_maps = []
    for i in range(NC):
        ib = slice(i * CB, (i + 1) * CB)
        in_maps.append({
            "X16": X16,
            "Xblk16": np.ascontiguousarray(X16[:, ib]),
            "V16": V16,
            "Vblk32": np.ascontiguousarray(V32[ib, :]),
        })

    res = run_bass_kernel_spmd(nc, in_maps, core_ids=list(range(NC)),
                               trace=trace)

    out = np.empty((B, N), np.float32)
    for i in range(NC):
        out[:, i * CB:(i + 1) * CB] = res.results[i]["acc_out"].T
    return out, res


def kernel(X, R, coeffs, t_mid, t_half):
    out, _ = _run(X, R, coeffs, t_mid, t_half, trace=False)
    return out
